# revision 3
# baseline (speedup 1.0000x reference)
"""BatchHardTripletLoss (with faithful source bug) on 8 Trainium2 NeuronCores.

Reference semantics (N=8192, D=128, C=10 classes, margin=1.0):
    d(i,j)   = max(x2_i + x2_j - 2 e_i.e_j, 0)
    d_pos[i] = max_{j: same class} d(i,j)                  (includes self)
    S[i,k]   = sum_{j: class k} d(i,j);  k* = argmax_k S[i,k]
    j*       = (k*)-th negative of i in (class, index) order
    loss     = mean relu(d_pos - d(i,j*) + 1)

Key structure exploited (validated against the reference, ~1e-5 rel):
  * Only the diagonal of d clamps at 0, and the diagonal is exactly 0, so S
    has the closed form S[i,k] = cnt_k*x2_i + C_k - 2 e_i.E_k.
  * k* < 10 <= class sizes, so j* is among the first 10 members of class 0
    (anchors with label != 0) or of class 1 (anchors with label == 0).
  * d_pos only needs distances within the anchor's own class block.

Device layout (v2 -- per-width slot profile):
  * The DVE is the only engine that can max-reduce PSUM; it runs at 1 fp32
    col/cycle @0.96GHz ((W+151)/0.96 ns per pass), so total DVE columns is
    the roofline.  Instead of 10 uniform slots of global-max width (8580
    cols/core), each core runs 9 static slots: 7 "home" tiles at
    W1 = max width of the 8 SMALLEST classes, and 2 "leftover" tiles at
    W2 = max width of the 2 LARGEST classes (those two classes pay only 2
    slots instead of 7).  7*W1 + 2*W2 ~= 7650 cols/core.
  * Home classes = 8 smallest, one per core (window DMA stays 2 windows
    per core).  Leftover class A -> cores 0..3, B -> cores 4..7, two tiles
    each; slots that exceed the real tile count replay tile 0 with
    hd = PAD_NEG so they contribute exactly 0 to the loss.
  * Tiles 0 and 1 are split into two half-width chunks with SEPARATE psum
    tiles so each DVE chunk waits only on its own matmul + DMA piece
    (a full-tile psum gets a conservative wait on both matmuls); the
    running max chains through the pass's accum seed (s0).
  * Division of labor: device does the O(N*cnt*D) window matmuls + the
    fused ADD_MAX_REDUCE / LOSS_SUM DVE passes; the hardest-negative
    mining is O(N*C*D) on host-resident stats (S[i,k] from per-class
    sums), shipped as hd[i] = x2_i - d_neg_i + margin.  The final
    128-partition sum also moved to the host: the kernel DMAs out the
    [128,1] per-partition loss sums, deleting the 1x1 partition-sum
    matmul + psum->sbuf copy from the critical tail.
  * Inputs ride 6 dma_start doorbells over the 2 HWDGE queues (SP+ACT),
    sized/ordered so the first DVE chunk starts ~doorbell+2.1us+one
    424-col matmul after the window opens, and each consumer waits only
    on the bytes it needs (a doorbell costs ~650ns engine time; a piece
    is consumable ~2.1us after its doorbell ends).
  * gpsimd runs nothing but the ones memset (any tensor op triggers a
    ~9us Q7 library load); the dummy 1x1 matmul absorbs the PE
    sequencer's ~2us first-instruction overhead during the DMA wait.
  * ~10us of every execution is fixed NEFF boilerplate (per-engine
    semaphore parade ~57 ops/engine + barriers, emitted by the
    runtime/walrus wrapper for any kernel on this stack).
"""

import numpy as np
from contextlib import ExitStack

import ml_dtypes
import concourse.bass as bass
import concourse.tile as tile
from concourse import bacc, mybir
from concourse import dve_ops
from concourse.dve_spec import (Spec, Src0, Src1, C0, maxx, relu, lower,
                                _has_src1, AluOp as DveAluOp)
from concourse.dve_uop import DveOpSpec
from concourse.bass_utils import run_bass_kernel_spmd

N_CORES = 8
C = 10
MARGIN = 1.0
P = 128
F32 = mybir.dt.float32
BF16 = mybir.dt.bfloat16
AX = mybir.AxisListType.X
ALU = mybir.AluOpType
NEG_INF = -3.0e38
PAD_NEG = -1.0e30

# stash of the last BassKernelResults (read by test.py for profiling)
last_results = None
_trace_opts: dict = {}


def _ref_add_max_reduce(in0, in1, c0, c1, c2):
    b = (np.asarray(in0, np.float32) + np.asarray(in1, np.float32))
    if isinstance(c0, np.ndarray):
        seed = np.asarray(c0, np.float32).reshape(-1, 1)
    else:
        seed = np.full((b.shape[0], 1), float(c0), np.float32)
    acc = np.maximum(seed, b.reshape(b.shape[0], -1).max(axis=-1, keepdims=True))
    return b.astype(np.float32), acc.astype(np.float32)


def _register_custom(name, spec):
    for op in dve_ops.OPS:
        if op.name == name:
            return op
    row = dve_ops._CUSTOM_DVE_ROW_BASE + len(dve_ops.OPS)
    assert row < 0x20
    dve_ops._SUB_OPCODE_FOR_NAME[name] = row
    shas = {}
    for ver in ("v3", "v4"):
        try:
            u = lower(spec, ver=ver)
            shas[ver] = DveOpSpec(name=name, opcode=row, uops=u,
                                  rd1_en=_has_src1(spec)).sha(ver)
        except Exception:
            pass
    assert shas, f"{name} failed to lower for any DVE version"
    op = dve_ops.DveOp(name, spec, subdim=False, uops_sha=shas)
    dve_ops.OPS.append(op)
    dve_ops.CUSTOM_DVE_SPECS[name] = spec
    return op


# out = in0 + in1; accum_out = max(s0, rowmax(out)).  Fuses the x2_j
# broadcast add into the hardest-positive max so each PSUM distance tile is
# consumed in a single DVE pass (native TENSOR_TENSOR_REDUCE hard-faults on
# this runtime).
ADD_MAX_REDUCE = _register_custom(
    "ADD_MAX_REDUCE_BHTL",
    Spec(body=Src0 + Src1, accum=maxx, accum_init=C0,
         reference=_ref_add_max_reduce))


def _ref_loss_sum(in0, in1, c0, c1, c2):
    b = np.maximum(np.asarray(in0, np.float32) + np.asarray(in1, np.float32)
                   + np.float32(c0), 0.0)
    acc = b.reshape(b.shape[0], -1).sum(axis=-1, keepdims=True)
    return b.astype(np.float32), acc.astype(np.float32)


# out = relu(in0 + in1 + c0); accum_out = rowsum(out).  Fuses the final
# margin-relu and the per-partition loss sum into one DVE pass (in1 is the
# NEGATED d_neg, shipped as hd = x2_i - d_neg + margin).
LOSS_SUM = _register_custom(
    "LOSS_SUM_BHTL",
    Spec(body=relu(Src0 + Src1 + C0), accum=DveAluOp.ADD,
         reference=_ref_loss_sum))


def _build_program(Q: int, NTH: int, W1: int, W2: int):
    """One SPMD program; all per-core variation is in the input tensors.

    Q slots per core: slots [0, NTH) process the core's home-class window
    (width W1), slots [NTH, Q) the core's leftover-class window (width W2).
    Tiles 0 and 1 are chunked in halves (WhA | W1-WhA) with separate psum
    tiles for fine-grained DMA/matmul deps during rampup.
    """
    nc = bacc.Bacc("TRN2", target_bir_lowering=False, debug=False,
                   num_devices=N_CORES)

    WhA = (W1 // 2) & ~1          # chunk-A width for tiles 0 and 1
    WhB = W1 - WhA
    # big0 (SP q):  [ a0 | a1 | w1a | w1b | a2 | a3..a8 ]
    #   P1=[a0|a1|w1a]  P2=[w1b|a2]  P3=[a3..]
    # big1 (ACT q): [ x2j1a | x2j1b | x2j2 | w2 | hd ]
    #   R1=[x2j1a]  R2=[x2j1b]  R3=[x2j2|w2|hd]
    n_big0 = Q * P + W1
    n_big1 = W1 + 2 * W2 + 2 * Q
    big0_d = nc.dram_tensor("big0", [P, n_big0], BF16, kind="ExternalInput").ap()
    big1_d = nc.dram_tensor("big1", [P, n_big1], BF16, kind="ExternalInput").ap()
    out_d = nc.dram_tensor("out", [P, 1], F32, kind="ExternalOutput").ap()

    # big0 column offsets
    O_A0, O_A1 = 0, P
    O_W1A = 2 * P
    O_W1B = O_W1A + WhA
    O_A2 = 2 * P + W1
    O_A3 = O_A2 + P
    # big1 column offsets
    O_XJ1, O_XJ2, O_W2, O_HD = 0, W1, W1 + W2, W1 + 2 * W2

    with tile.TileContext(nc) as tc, ExitStack() as ctx:
        const = ctx.enter_context(tc.tile_pool(name="const", bufs=1))
        psum = ctx.enter_context(tc.tile_pool(name="psum", bufs=3, space="PSUM"))
        psc = ctx.enter_context(tc.tile_pool(name="psc", bufs=1, space="PSUM"))
        scratch = ctx.enter_context(tc.tile_pool(name="scratch", bufs=2))

        ones_sb = const.tile([P, 1], F32)
        nc.gpsimd.memset(ones_sb[:], 1.0)
        # dummy 1x1 matmul: absorbs the PE sequencer's ~2us first-instruction
        # overhead while the input DMAs are still in flight
        psd = psc.tile([1, 1], F32, tag="pv", name="psd")
        nc.tensor.matmul(psd[:], ones_sb[:], ones_sb[:], start=True, stop=True)

        big0_sb = const.tile([P, n_big0], BF16)
        big1_sb = const.tile([P, n_big1], BF16)
        # piece emission order == doorbell order per queue engine
        nc.sync.dma_start(big0_sb[:, 0:O_W1B], big0_d[:, 0:O_W1B])      # P1
        nc.scalar.dma_start(big1_sb[:, O_XJ1:O_XJ1 + WhA],
                            big1_d[:, O_XJ1:O_XJ1 + WhA])               # R1
        nc.sync.dma_start(big0_sb[:, O_W1B:O_A3], big0_d[:, O_W1B:O_A3])  # P2
        nc.scalar.dma_start(big1_sb[:, O_XJ1 + WhA:O_XJ1 + W1],
                            big1_d[:, O_XJ1 + WhA:O_XJ1 + W1])          # R2
        nc.sync.dma_start(big0_sb[:, O_A3:], big0_d[:, O_A3:])          # P3
        nc.scalar.dma_start(big1_sb[:, O_XJ2:], big1_d[:, O_XJ2:])      # R3

        x2j1 = big1_sb[:, O_XJ1:O_XJ1 + W1]
        x2j2 = big1_sb[:, O_XJ2:O_XJ2 + W2]

        mall = const.tile([P, Q], F32)         # max_j(x2_j - 2 e_i.e_j)

        def anchor(t):
            if t == 0:
                return big0_sb[:, O_A0:O_A0 + P]
            if t == 1:
                return big0_sb[:, O_A1:O_A1 + P]
            if t == 2:
                return big0_sb[:, O_A2:O_A2 + P]
            return big0_sb[:, O_A3 + (t - 3) * P:O_A3 + (t - 2) * P]

        for t in range(Q):
            lhs = anchor(t)
            if t < 2:
                # chunked: separate psum tiles -> exact per-chunk deps
                psA = psum.tile([P, WhA], F32, tag="ps", name=f"ps{t}a")
                psB = psum.tile([P, WhB], F32, tag="ps", name=f"ps{t}b")
                nc.tensor.matmul(psA[:], lhs, big0_sb[:, O_W1A:O_W1A + WhA],
                                 start=True, stop=True)
                nc.tensor.matmul(psB[:], lhs, big0_sb[:, O_W1B:O_W1B + WhB],
                                 start=True, stop=True)
                dsc = scratch.tile([P, W1], F32)
                nc.vector._custom_dve(ADD_MAX_REDUCE, out=dsc[:, 0:WhA],
                                      in0=psA[:], in1=x2j1[:, 0:WhA],
                                      s0=NEG_INF, accum_out=mall[:, t:t + 1])
                nc.vector._custom_dve(ADD_MAX_REDUCE, out=dsc[:, WhA:W1],
                                      in0=psB[:], in1=x2j1[:, WhA:W1],
                                      s0=mall[:, t:t + 1],
                                      accum_out=mall[:, t:t + 1])
                continue
            if t < NTH:
                W, w, xj = W1, big0_sb[:, O_W1A:O_W1A + W1], x2j1
            else:
                W, w, xj = W2, big1_sb[:, O_W2:O_W2 + W2], x2j2
            ps = psum.tile([P, W], F32, tag="ps", name=f"ps{t}")
            nc.tensor.matmul(ps[:, 0:512], lhs, w[:, 0:512],
                             start=True, stop=True)
            nc.tensor.matmul(ps[:, 512:W], lhs, w[:, 512:W],
                             start=True, stop=True)
            dsc = scratch.tile([P, W], F32)
            nc.vector._custom_dve(ADD_MAX_REDUCE, out=dsc[:],
                                  in0=ps[:], in1=xj,
                                  s0=NEG_INF, accum_out=mall[:, t:t + 1])

        # loss = relu(mall + hd) summed per partition, one fused DVE pass;
        # the 128-partition reduction happens on the host (out is [128,1])
        hd_f = big1_sb[:, O_HD:O_HD + 2 * Q].bitcast(F32)
        t3 = const.tile([P, Q], F32)
        lsum = const.tile([P, 1], F32)
        nc.vector._custom_dve(LOSS_SUM, out=t3[:], in0=mall[:], in1=hd_f,
                              s0=0.0, accum_out=lsum[:])
        nc.sync.dma_start(out_d[:], lsum[:])

    nc.compile()
    return nc


_prog_cache: dict = {}


def kernel(embeddings: np.ndarray, labels: np.ndarray) -> np.ndarray:
    global last_results
    e = np.ascontiguousarray(np.asarray(embeddings), dtype=np.float32)
    lab = np.asarray(labels).astype(np.int64)
    N, D = e.shape
    assert D == P and N % N_CORES == 0

    # ---- host-side marshalling: class-sort, per-class stats ----
    order = np.argsort(lab * N + np.arange(N))
    e = e[order]
    lab_s = lab[order]
    cnt = np.bincount(lab_s, minlength=C)
    assert len(cnt) == C and cnt[0] >= 10 and cnt[1] >= 10, cnt
    offs = np.zeros(C + 1, dtype=np.int64)
    offs[1:] = np.cumsum(cnt)

    x2 = np.einsum("nd,nd->n", e, e).astype(np.float32)
    E = np.stack([e[offs[k]:offs[k + 1]].sum(axis=0) for k in range(C)],
                 axis=1).astype(np.float32)          # [D, C]
    Ck = np.array([x2[offs[k]:offs[k + 1]].sum() for k in range(C)],
                  dtype=np.float32)                  # [C]
    candA = e[0:10]                                  # class-0 members
    candB = e[offs[1]:offs[1] + 10]                  # class-1 members
    x2A, x2B = x2[0:10], x2[offs[1]:offs[1] + 10]
    cnt_f = cnt.astype(np.float32)

    # ---- slot profile: homes = 8 smallest classes, leftovers = 2 largest
    by_w = np.argsort(cnt, kind="stable")            # asc
    homes = [int(k) for k in by_w[:8]]
    lo = [int(k) for k in by_w[8:]]                  # 2 largest
    W1 = int(max(cnt[k] for k in homes));  W1 += W1 & 1
    W2 = int(max(cnt[k] for k in lo));     W2 += W2 & 1
    NTH = -(-W1 // P)                                # home anchor tiles
    NTL = -(-int(max(cnt[k] for k in lo)) // P)      # real leftover tiles
    L = -(-2 * NTL // 8)                             # leftover slots/core
    Q = NTH + L
    assert W1 > 512 and W2 > 512 and NTH >= 2

    # per-class padded member blocks (pad rows/cols duplicate member 0 --
    # they never win a max; pad anchor rows are squashed via hd = PAD_NEG)
    def padded(k, nrows):
        m = int(cnt[k])
        blk = np.empty((nrows, D), np.float32)
        blk[:m] = e[offs[k]:offs[k + 1]]
        blk[m:] = e[offs[k]]
        xx = np.empty(nrows, np.float32)
        xx[:m] = x2[offs[k]:offs[k + 1]]
        xx[m:] = x2[offs[k]]
        vv = np.zeros(nrows, np.float32)
        vv[:m] = 1.0
        return blk, xx, vv

    key = (Q, NTH, W1, W2)
    if key not in _prog_cache:
        _prog_cache[key] = _build_program(Q, NTH, W1, W2)
    nc = _prog_cache[key]

    def mine_hd(ei, xi, vm, klab):
        # hardest-negative mining from per-class stats (host O(P*C*D)):
        # S[i,k] = cnt_k*x2_i + C_k - 2 e_i.E_k, k* = argmax_k S, then
        # hd = x2_i - max(d(i, cand[k*]), 0) + margin
        cand = candB if klab == 0 else candA
        x2c = x2B if klab == 0 else x2A
        St = xi[:, None] * cnt_f[None, :] + Ck[None, :] - 2.0 * (ei @ E)
        ks = St.argmax(axis=1)
        dn = xi + x2c[ks] - 2.0 * np.einsum("nd,nd->n", ei, cand[ks])
        return np.where(vm > 0.5, xi - np.maximum(dn, 0.0) + MARGIN, PAD_NEG)

    in_maps = []
    for c in range(N_CORES):
        hk = homes[c]
        lk = lo[0] if c < N_CORES // 2 else lo[1]
        ci = c if c < N_CORES // 2 else c - N_CORES // 2

        hblk, hx2, hval = padded(hk, NTH * P)
        lblk, lx2, lval = padded(lk, NTL * P)
        w1blk, w1x2, _ = padded(hk, W1)              # window cols (pad dup)
        w2blk, w2x2, _ = padded(lk, W2)

        # anchors: NTH home tiles + L leftover tiles (filler replays tile 0)
        anch = np.empty((Q * P, D), np.float32)
        hd = np.empty((P, Q), np.float32)
        anch[:NTH * P] = hblk
        for t in range(NTH):
            sl = slice(t * P, (t + 1) * P)
            hd[:, t] = mine_hd(hblk[sl], hx2[sl], hval[sl], hk)
        for j in range(L):
            t = NTH + j
            idx = ci * L + j
            if idx < NTL:
                sl = slice(idx * P, (idx + 1) * P)
                anch[t * P:(t + 1) * P] = lblk[sl]
                hd[:, t] = mine_hd(lblk[sl], lx2[sl], lval[sl], lk)
            else:                                    # filler slot
                anch[t * P:(t + 1) * P] = lblk[0:P]
                hd[:, t] = PAD_NEG

        a = (-2.0 * anch.T).astype(ml_dtypes.bfloat16)   # [D, Q*128]
        w1 = w1blk.T.astype(ml_dtypes.bfloat16)          # [D, W1]
        w2 = w2blk.T.astype(ml_dtypes.bfloat16)          # [D, W2]
        x2j1 = np.broadcast_to(
            w1x2[None, :].astype(ml_dtypes.bfloat16), (P, W1))
        x2j2 = np.broadcast_to(
            w2x2[None, :].astype(ml_dtypes.bfloat16), (P, W2))

        big0 = np.concatenate([
            a[:, 0:2 * P],                 # a0 a1
            w1,                            # w1a|w1b
            a[:, 2 * P:Q * P],             # a2..
        ], axis=1)
        big1 = np.concatenate([
            x2j1, x2j2, w2,
            np.ascontiguousarray(hd).view(ml_dtypes.bfloat16),
        ], axis=1)
        in_maps.append({"big0": big0, "big1": big1})

    res = run_bass_kernel_spmd(nc, in_maps, list(range(N_CORES)), **_trace_opts)
    last_results = res
    total = np.float64(0.0)
    for c in range(N_CORES):
        total += res.results[c]["out"].astype(np.float64).sum()
    return np.asarray(total / N, dtype=np.float32)


# revision 9
# speedup vs baseline: 1.2315x; 1.2315x over previous
"""BatchHardTripletLoss (with faithful source bug) on 8 Trainium2 NeuronCores.

Reference semantics (N=8192, D=128, C=10 classes, margin=1.0):
    d(i,j)   = max(x2_i + x2_j - 2 e_i.e_j, 0)
    d_pos[i] = max_{j: same class} d(i,j)                  (includes self)
    S[i,k]   = sum_{j: class k} d(i,j);  k* = argmax_k S[i,k]
    j*       = (k*)-th negative of i in (class, index) order
    loss     = mean relu(d_pos - d(i,j*) + 1)

Key structure exploited (validated against the reference, ~1e-5 rel):
  * Only the diagonal of d clamps at 0, and the diagonal is exactly 0, so S
    has the closed form S[i,k] = cnt_k*x2_i + C_k - 2 e_i.E_k.
  * k* < 10 <= class sizes, so j* is among the first 10 members of class 0
    (anchors with label != 0) or of class 1 (anchors with label == 0).
  * d_pos only needs distances within the anchor's own class block.

Device layout (v2 -- per-width slot profile):
  * The DVE is the only engine that can max-reduce PSUM; it runs at 1 fp32
    col/cycle @0.96GHz ((W+151)/0.96 ns per pass), so total DVE columns is
    the roofline.  Instead of 10 uniform slots of global-max width (8580
    cols/core), each core runs 9 static slots: 7 "home" tiles at
    W1 = max width of the 8 SMALLEST classes, and 2 "leftover" tiles at
    W2 = max width of the 2 LARGEST classes (those two classes pay only 2
    slots instead of 7).  7*W1 + 2*W2 ~= 7650 cols/core.
  * Home classes = 8 smallest, one per core (window DMA stays 2 windows
    per core).  Leftover class A -> cores 0..3, B -> cores 4..7, two tiles
    each; slots that exceed the real tile count replay tile 0 with
    hd = PAD_NEG so they contribute exactly 0 to the loss.
  * Tiles 0 and 1 are split into two half-width chunks with SEPARATE psum
    tiles so each DVE chunk waits only on its own matmul + DMA piece
    (a full-tile psum gets a conservative wait on both matmuls); the
    running max chains through the pass's accum seed (s0).
  * Division of labor: device does the O(N*cnt*D) window matmuls + the
    fused ADD_MAX_REDUCE / LOSS_SUM DVE passes; the hardest-negative
    mining is O(N*C*D) on host-resident stats (S[i,k] from per-class
    sums), shipped as hd[i] = x2_i - d_neg_i + margin.  The final
    128-partition sum also moved to the host: the kernel DMAs out the
    [128,1] per-partition loss sums, deleting the 1x1 partition-sum
    matmul + psum->sbuf copy from the critical tail.
  * Inputs ride 6 dma_start doorbells over the 2 HWDGE queues (SP+ACT),
    sized/ordered so the first DVE chunk starts ~doorbell+2.1us+one
    424-col matmul after the window opens, and each consumer waits only
    on the bytes it needs (a doorbell costs ~650ns engine time; a piece
    is consumable ~2.1us after its doorbell ends).
  * gpsimd runs nothing but the ones memset (any tensor op triggers a
    ~9us Q7 library load); the dummy 1x1 matmul absorbs the PE
    sequencer's ~2us first-instruction overhead during the DMA wait.
  * ~10us of every execution is fixed NEFF boilerplate (per-engine
    semaphore parade ~57 ops/engine + barriers, emitted by the
    runtime/walrus wrapper for any kernel on this stack).
"""

import numpy as np
from contextlib import ExitStack

import ml_dtypes
import concourse.bass as bass
import concourse.tile as tile
from concourse import bacc, mybir
from concourse import dve_ops
from concourse.dve_spec import (Spec, Src0, Src1, C0, maxx, relu, lower,
                                _has_src1, AluOp as DveAluOp)
from concourse.dve_uop import DveOpSpec
from concourse.bass_utils import run_bass_kernel_spmd

N_CORES = 8
C = 10
MARGIN = 1.0
P = 128
F32 = mybir.dt.float32
BF16 = mybir.dt.bfloat16
AX = mybir.AxisListType.X
ALU = mybir.AluOpType
NEG_INF = -3.0e38
PAD_NEG = -1.0e30

# stash of the last BassKernelResults (read by test.py for profiling)
last_results = None
_trace_opts: dict = {}


def _ref_add_max_reduce(in0, in1, c0, c1, c2):
    b = (np.asarray(in0, np.float32) + np.asarray(in1, np.float32))
    if isinstance(c0, np.ndarray):
        seed = np.asarray(c0, np.float32).reshape(-1, 1)
    else:
        seed = np.full((b.shape[0], 1), float(c0), np.float32)
    acc = np.maximum(seed, b.reshape(b.shape[0], -1).max(axis=-1, keepdims=True))
    return b.astype(np.float32), acc.astype(np.float32)


def _register_custom(name, spec):
    for op in dve_ops.OPS:
        if op.name == name:
            return op
    row = dve_ops._CUSTOM_DVE_ROW_BASE + len(dve_ops.OPS)
    assert row < 0x20
    dve_ops._SUB_OPCODE_FOR_NAME[name] = row
    shas = {}
    for ver in ("v3", "v4"):
        try:
            u = lower(spec, ver=ver)
            shas[ver] = DveOpSpec(name=name, opcode=row, uops=u,
                                  rd1_en=_has_src1(spec)).sha(ver)
        except Exception:
            pass
    assert shas, f"{name} failed to lower for any DVE version"
    op = dve_ops.DveOp(name, spec, subdim=False, uops_sha=shas)
    dve_ops.OPS.append(op)
    dve_ops.CUSTOM_DVE_SPECS[name] = spec
    return op


# out = in0 + in1; accum_out = max(s0, rowmax(out)).  Fuses the x2_j
# broadcast add into the hardest-positive max so each PSUM distance tile is
# consumed in a single DVE pass (native TENSOR_TENSOR_REDUCE hard-faults on
# this runtime).
ADD_MAX_REDUCE = _register_custom(
    "ADD_MAX_REDUCE_BHTL",
    Spec(body=Src0 + Src1, accum=maxx, accum_init=C0,
         reference=_ref_add_max_reduce))


def _ref_loss_sum(in0, in1, c0, c1, c2):
    b = np.maximum(np.asarray(in0, np.float32) + np.asarray(in1, np.float32)
                   + np.float32(c0), 0.0)
    acc = b.reshape(b.shape[0], -1).sum(axis=-1, keepdims=True)
    return b.astype(np.float32), acc.astype(np.float32)


# out = relu(in0 + in1 + c0); accum_out = rowsum(out).  Fuses the final
# margin-relu and the per-partition loss sum into one DVE pass (in1 is the
# NEGATED d_neg, shipped as hd = x2_i - d_neg + margin).
LOSS_SUM = _register_custom(
    "LOSS_SUM_BHTL",
    Spec(body=relu(Src0 + Src1 + C0), accum=DveAluOp.ADD,
         reference=_ref_loss_sum))


def _build_program(Q: int, NTH: int, W1: int, W2: int):
    """One SPMD program; all per-core variation is in the input tensors.

    Q slots per core: slots [0, NTH) process the core's home-class window
    (width W1), slots [NTH, Q) the core's leftover-class window (width W2).
    Tiles 0 and 1 are chunked in halves (WhA | W1-WhA) with separate psum
    tiles for fine-grained DMA/matmul deps during rampup.
    """
    nc = bacc.Bacc("TRN2", target_bir_lowering=False, debug=False,
                   num_devices=N_CORES)

    WhA = (W1 // 2) & ~1          # chunk-A width for tiles 0 and 1
    WhB = W1 - WhA
    # DMA economics: a piece costs ~1.3us of its queue (row-overhead bound,
    # ~163ns/row/ring) regardless of width, and pieces FIFO per queue -- so
    # each queue carries one minimal stream-opening piece, one piece for the
    # second chunk wave, and one wide piece with everything else.
    # big0 (SP q):  [ a0 | w1a | w1b | a1 | a2..a8 ]
    #   P1=[a0|w1a]  P2=[w1b|a1]  P3=[a2..]
    # big1 (ACT q): [ x2j1a | x2j1b | x2j2 | w2 | hd ]
    #   R1=[x2j1a]  R2=[x2j1b]  R3=[x2j2|w2|hd]
    n_big0 = Q * P + W1
    n_big1 = W1 + 2 * W2 + 2 * Q
    big0_d = nc.dram_tensor("big0", [P, n_big0], BF16, kind="ExternalInput").ap()
    big1_d = nc.dram_tensor("big1", [P, n_big1], BF16, kind="ExternalInput").ap()
    out_d = nc.dram_tensor("out", [1, 1], F32, kind="ExternalOutput").ap()

    # big0 column offsets
    O_A0 = 0
    O_W1A = P
    O_W1B = O_W1A + WhA
    O_A1 = P + W1
    O_A2 = O_A1 + P
    # big1 column offsets
    O_XJ1, O_XJ2, O_W2, O_HD = 0, W1, W1 + W2, W1 + 2 * W2

    with tile.TileContext(nc) as tc, ExitStack() as ctx:
        const = ctx.enter_context(tc.tile_pool(name="const", bufs=1))
        psum = ctx.enter_context(tc.tile_pool(name="psum", bufs=3, space="PSUM"))
        psc = ctx.enter_context(tc.tile_pool(name="psc", bufs=2, space="PSUM"))
        scratch = ctx.enter_context(tc.tile_pool(name="scratch", bufs=2))

        ones_sb = const.tile([P, 1], F32)
        nc.gpsimd.memset(ones_sb[:], 1.0)
        # dummy 1x1 matmul: absorbs the PE sequencer's ~2us first-instruction
        # overhead while the input DMAs are still in flight
        psd = psc.tile([1, 1], F32, tag="pv", name="psd")
        nc.tensor.matmul(psd[:], ones_sb[:], ones_sb[:], start=True, stop=True)

        big0_sb = const.tile([P, n_big0], BF16)
        big1_sb = const.tile([P, n_big1], BF16)
        # piece emission order == doorbell order per queue engine
        nc.sync.dma_start(big0_sb[:, 0:O_W1B], big0_d[:, 0:O_W1B])      # P1
        nc.scalar.dma_start(big1_sb[:, O_XJ1:O_XJ1 + WhA],
                            big1_d[:, O_XJ1:O_XJ1 + WhA])               # R1
        nc.sync.dma_start(big0_sb[:, O_W1B:O_A2], big0_d[:, O_W1B:O_A2])  # P2
        nc.scalar.dma_start(big1_sb[:, O_XJ1 + WhA:O_XJ1 + W1],
                            big1_d[:, O_XJ1 + WhA:O_XJ1 + W1])          # R2
        nc.sync.dma_start(big0_sb[:, O_A2:], big0_d[:, O_A2:])          # P3
        nc.scalar.dma_start(big1_sb[:, O_XJ2:], big1_d[:, O_XJ2:])      # R3

        x2j1 = big1_sb[:, O_XJ1:O_XJ1 + W1]
        x2j2 = big1_sb[:, O_XJ2:O_XJ2 + W2]

        mall = const.tile([P, Q], F32)         # max_j(x2_j - 2 e_i.e_j)

        def anchor(t):
            if t == 0:
                return big0_sb[:, O_A0:O_A0 + P]
            if t == 1:
                return big0_sb[:, O_A1:O_A1 + P]
            return big0_sb[:, O_A2 + (t - 2) * P:O_A2 + (t - 1) * P]

        for t in range(Q):
            lhs = anchor(t)
            if t < 2:
                # chunked: separate psum tiles -> exact per-chunk deps
                psA = psum.tile([P, WhA], F32, tag="ps", name=f"ps{t}a")
                psB = psum.tile([P, WhB], F32, tag="ps", name=f"ps{t}b")
                nc.tensor.matmul(psA[:], lhs, big0_sb[:, O_W1A:O_W1A + WhA],
                                 start=True, stop=True)
                nc.tensor.matmul(psB[:], lhs, big0_sb[:, O_W1B:O_W1B + WhB],
                                 start=True, stop=True)
                dsc = scratch.tile([P, W1], F32)
                nc.vector._custom_dve(ADD_MAX_REDUCE, out=dsc[:, 0:WhA],
                                      in0=psA[:], in1=x2j1[:, 0:WhA],
                                      s0=NEG_INF, accum_out=mall[:, t:t + 1])
                nc.vector._custom_dve(ADD_MAX_REDUCE, out=dsc[:, WhA:W1],
                                      in0=psB[:], in1=x2j1[:, WhA:W1],
                                      s0=mall[:, t:t + 1],
                                      accum_out=mall[:, t:t + 1])
                continue
            if t < NTH:
                W, w, xj = W1, big0_sb[:, O_W1A:O_W1A + W1], x2j1
            else:
                W, w, xj = W2, big1_sb[:, O_W2:O_W2 + W2], x2j2
            ps = psum.tile([P, W], F32, tag="ps", name=f"ps{t}")
            nc.tensor.matmul(ps[:, 0:512], lhs, w[:, 0:512],
                             start=True, stop=True)
            nc.tensor.matmul(ps[:, 512:W], lhs, w[:, 512:W],
                             start=True, stop=True)
            dsc = scratch.tile([P, W], F32)
            nc.vector._custom_dve(ADD_MAX_REDUCE, out=dsc[:],
                                  in0=ps[:], in1=xj,
                                  s0=NEG_INF, accum_out=mall[:, t:t + 1])

        # loss = relu(mall + hd) summed per partition, one fused DVE pass
        hd_f = big1_sb[:, O_HD:O_HD + 2 * Q].bitcast(F32)
        t3 = const.tile([P, Q], F32)
        lsum = const.tile([P, 1], F32)
        nc.vector._custom_dve(LOSS_SUM, out=t3[:], in0=mall[:], in1=hd_f,
                              s0=0.0, accum_out=lsum[:])
        # partition-sum via a 1-column matmul so the output DMA is a single
        # 4-byte transfer (a [128,1] out-DMA costs ~9us: 128 tiny rows)
        pout = psc.tile([1, 1], F32, tag="pv")
        nc.tensor.matmul(pout[:], lsum[:], ones_sb[:], start=True, stop=True)
        res_sb = const.tile([1, 1], F32)
        nc.vector.tensor_scalar(res_sb[:], pout[:], 0.0, NEG_INF,
                                op0=ALU.add, op1=ALU.max)
        nc.sync.dma_start(out_d[:], res_sb[:])

    nc.compile()
    return nc


_prog_cache: dict = {}


def kernel(embeddings: np.ndarray, labels: np.ndarray) -> np.ndarray:
    global last_results
    e = np.ascontiguousarray(np.asarray(embeddings), dtype=np.float32)
    lab = np.asarray(labels).astype(np.int64)
    N, D = e.shape
    assert D == P and N % N_CORES == 0

    # ---- host-side marshalling: class-sort, per-class stats ----
    order = np.argsort(lab * N + np.arange(N))
    e = e[order]
    lab_s = lab[order]
    cnt = np.bincount(lab_s, minlength=C)
    assert len(cnt) == C and cnt[0] >= 10 and cnt[1] >= 10, cnt
    offs = np.zeros(C + 1, dtype=np.int64)
    offs[1:] = np.cumsum(cnt)

    x2 = np.einsum("nd,nd->n", e, e).astype(np.float32)
    E = np.stack([e[offs[k]:offs[k + 1]].sum(axis=0) for k in range(C)],
                 axis=1).astype(np.float32)          # [D, C]
    Ck = np.array([x2[offs[k]:offs[k + 1]].sum() for k in range(C)],
                  dtype=np.float32)                  # [C]
    candA = e[0:10]                                  # class-0 members
    candB = e[offs[1]:offs[1] + 10]                  # class-1 members
    x2A, x2B = x2[0:10], x2[offs[1]:offs[1] + 10]
    cnt_f = cnt.astype(np.float32)

    # ---- slot profile: homes = 8 smallest classes, leftovers = 2 largest
    by_w = np.argsort(cnt, kind="stable")            # asc
    homes = [int(k) for k in by_w[:8]]
    lo = [int(k) for k in by_w[8:]]                  # 2 largest
    W1 = int(max(cnt[k] for k in homes));  W1 += W1 & 1
    W2 = int(max(cnt[k] for k in lo));     W2 += W2 & 1
    NTH = -(-W1 // P)                                # home anchor tiles
    NTL = -(-int(max(cnt[k] for k in lo)) // P)      # real leftover tiles
    L = -(-2 * NTL // 8)                             # leftover slots/core
    Q = NTH + L
    assert W1 > 512 and W2 > 512 and NTH >= 2

    # per-class padded member blocks (pad rows/cols duplicate member 0 --
    # they never win a max; pad anchor rows are squashed via hd = PAD_NEG)
    def padded(k, nrows):
        m = int(cnt[k])
        blk = np.empty((nrows, D), np.float32)
        blk[:m] = e[offs[k]:offs[k + 1]]
        blk[m:] = e[offs[k]]
        xx = np.empty(nrows, np.float32)
        xx[:m] = x2[offs[k]:offs[k + 1]]
        xx[m:] = x2[offs[k]]
        vv = np.zeros(nrows, np.float32)
        vv[:m] = 1.0
        return blk, xx, vv

    key = (Q, NTH, W1, W2)
    if key not in _prog_cache:
        _prog_cache[key] = _build_program(Q, NTH, W1, W2)
    nc = _prog_cache[key]

    def mine_hd(ei, xi, vm, klab):
        # hardest-negative mining from per-class stats (host O(P*C*D)):
        # S[i,k] = cnt_k*x2_i + C_k - 2 e_i.E_k, k* = argmax_k S, then
        # hd = x2_i - max(d(i, cand[k*]), 0) + margin
        cand = candB if klab == 0 else candA
        x2c = x2B if klab == 0 else x2A
        St = xi[:, None] * cnt_f[None, :] + Ck[None, :] - 2.0 * (ei @ E)
        ks = St.argmax(axis=1)
        dn = xi + x2c[ks] - 2.0 * np.einsum("nd,nd->n", ei, cand[ks])
        return np.where(vm > 0.5, xi - np.maximum(dn, 0.0) + MARGIN, PAD_NEG)

    in_maps = []
    for c in range(N_CORES):
        hk = homes[c]
        lk = lo[0] if c < N_CORES // 2 else lo[1]
        ci = c if c < N_CORES // 2 else c - N_CORES // 2

        hblk, hx2, hval = padded(hk, NTH * P)
        lblk, lx2, lval = padded(lk, NTL * P)
        w1blk, w1x2, _ = padded(hk, W1)              # window cols (pad dup)
        w2blk, w2x2, _ = padded(lk, W2)

        # anchors: NTH home tiles + L leftover tiles (filler replays tile 0)
        anch = np.empty((Q * P, D), np.float32)
        hd = np.empty((P, Q), np.float32)
        anch[:NTH * P] = hblk
        for t in range(NTH):
            sl = slice(t * P, (t + 1) * P)
            hd[:, t] = mine_hd(hblk[sl], hx2[sl], hval[sl], hk)
        for j in range(L):
            t = NTH + j
            idx = ci * L + j
            if idx < NTL:
                sl = slice(idx * P, (idx + 1) * P)
                anch[t * P:(t + 1) * P] = lblk[sl]
                hd[:, t] = mine_hd(lblk[sl], lx2[sl], lval[sl], lk)
            else:                                    # filler slot
                anch[t * P:(t + 1) * P] = lblk[0:P]
                hd[:, t] = PAD_NEG

        a = (-2.0 * anch.T).astype(ml_dtypes.bfloat16)   # [D, Q*128]
        w1 = w1blk.T.astype(ml_dtypes.bfloat16)          # [D, W1]
        w2 = w2blk.T.astype(ml_dtypes.bfloat16)          # [D, W2]
        x2j1 = np.broadcast_to(
            w1x2[None, :].astype(ml_dtypes.bfloat16), (P, W1))
        x2j2 = np.broadcast_to(
            w2x2[None, :].astype(ml_dtypes.bfloat16), (P, W2))

        big0 = np.concatenate([
            a[:, 0:P],                     # a0
            w1,                            # w1a|w1b
            a[:, P:Q * P],                 # a1 a2..
        ], axis=1)
        big1 = np.concatenate([
            x2j1, x2j2, w2,
            np.ascontiguousarray(hd).view(ml_dtypes.bfloat16),
        ], axis=1)
        in_maps.append({"big0": big0, "big1": big1})

    res = run_bass_kernel_spmd(nc, in_maps, list(range(N_CORES)), **_trace_opts)
    last_results = res
    total = np.float64(0.0)
    for c in range(N_CORES):
        total += res.results[c]["out"].astype(np.float64).sum()
    return np.asarray(total / N, dtype=np.float32)
